# revision 1
# baseline (speedup 1.0000x reference)
"""Trainium2 Bass kernel for GatedMultiScaleRetentionLayer.

Sharding: 8 cores = data-parallel over batch (B=2) x tensor-parallel over
heads (16 heads -> 4 groups of 4). Each core computes its batch's tokens for
its 4 heads end-to-end (QKV+gate projections on a 256-column weight slice,
xpos-rotary, decay-masked retention, per-head GroupNorm, silu gate, partial
output projection). Host sums the 4 partial outputs per batch and adds bo.

All heavy matmuls run in bf16 on the PE with fp32 PSUM accumulation. The
retention decay mask is folded into per-token scale tensors:
    qhat_i = rot(q_i) * gamma^i * rownorm_i ,  khat_j = rot(k_j) * gamma^-j
so masking reduces to causality (free for strictly-lower block pairs, one
[128,128] triangular multiply on diagonal blocks). The interleaved rotary is
made partition-friendly by permuting the rotary dims of Wq/Wk columns
(even indices first), so rotate_half becomes a fixed 16-row block swap done
with one small PE matmul against a +-1 permutation matrix.
"""

import os

import numpy as np
import ml_dtypes

import concourse.bass as bass
import concourse.tile as tile
from concourse import bacc, mybir
from concourse.bass_utils import run_bass_kernel_spmd

BF16 = ml_dtypes.bfloat16

# ---- problem constants (hardcoded per contract) ----
B = 2
S = 2048
D = 1024
H = 16
DH = 64
ROT = 32
THETA = 10000.0
XPOS_BASE = 512.0
GN_EPS = 1e-5

N_CORES = 8
HG = 4          # head groups (tensor-parallel)
HPC = 4         # heads per core
NCH = HPC * DH  # 256 channels per core
P = 128
QCH = 512       # qi chunk (one PSUM bank of fp32)
NQC = S // QCH  # 4 qi chunks
KBLK = 128      # kj block
NKB = S // KBLK # 16 kj blocks
NELEM = float(S * DH)  # groupnorm element count per head

LAST_EXEC_NS = None
LAST_RESULTS = None

_PERM = np.concatenate([np.arange(0, ROT, 2), np.arange(1, ROT, 2),
                        np.arange(ROT, DH)])  # within-head column permutation


def _perm_cols(w_slice):
    """Permute rotary dims of each head's 64-column block (even idx first)."""
    out = np.empty_like(w_slice)
    for hl in range(HPC):
        blk = w_slice[..., hl * DH:(hl + 1) * DH]
        out[..., hl * DH:(hl + 1) * DH] = blk[..., _PERM]
    return out


def _rot_tables():
    """angle[t, r], xpos scale[t, r] for pair index r in [0,16)."""
    t = np.arange(S, dtype=np.float64)
    r = np.arange(ROT // 2, dtype=np.float64)
    inv_freq = 1.0 / (THETA ** ((2.0 * r) / ROT))
    angle = t[:, None] * inv_freq[None, :]                   # [S, 16]
    base = (2.0 * r + 0.4 * ROT) / (1.4 * ROT)               # [16]
    power = (t - S // 2) / XPOS_BASE                         # [S]
    scale = base[None, :] ** power[:, None]                  # [S, 16]
    return angle, scale


def _decay_factors():
    """gamma^i*rownorm (for q) and gamma^-j (for k), per global head. f64."""
    h = np.arange(H, dtype=np.float64)
    gamma = 1.0 - 2.0 ** (-5.0 - h)                          # [H]
    t = np.arange(S, dtype=np.float64)
    logg = np.log(gamma)
    g_pos = np.exp(t[None, :] * logg[:, None])               # [H, S] gamma^t
    g_neg = np.exp(-t[None, :] * logg[:, None])              # [H, S] gamma^-t
    rowsum = (1.0 - gamma[:, None] * g_pos) / (1.0 - gamma[:, None])
    rownorm = 1.0 / np.sqrt(rowsum)                          # [H, S]
    return g_pos * rownorm, g_neg


def _cs_tiles(h0):
    """cq, sq, ck, sk tiles [2, 128, S] bf16 for heads h0..h0+3."""
    angle, scale = _rot_tables()
    dq_all, dk_all = _decay_factors()
    cos, sin = np.cos(angle), np.sin(angle)                  # [S, 16]
    cq = np.zeros((2, P, S), np.float64)
    sq = np.zeros((2, P, S), np.float64)
    ck = np.zeros((2, P, S), np.float64)
    sk = np.zeros((2, P, S), np.float64)
    for mt in range(2):
        for half in range(2):
            hl = 2 * mt + half
            g = h0 + hl
            dq = dq_all[g]                                   # [S]
            dk = dk_all[g]
            base = 64 * half
            for rr in range(16):
                cq[mt, base + rr] = cos[:, rr] * scale[:, rr] * dq
                cq[mt, base + 16 + rr] = cos[:, rr] * scale[:, rr] * dq
                sq[mt, base + rr] = sin[:, rr] * scale[:, rr] * dq
                sq[mt, base + 16 + rr] = sin[:, rr] * scale[:, rr] * dq
                ck[mt, base + rr] = cos[:, rr] / scale[:, rr] * dk
                ck[mt, base + 16 + rr] = cos[:, rr] / scale[:, rr] * dk
                sk[mt, base + rr] = sin[:, rr] / scale[:, rr] * dk
                sk[mt, base + 16 + rr] = sin[:, rr] / scale[:, rr] * dk
            cq[mt, base + 32:base + 64] = dq[None, :]
            ck[mt, base + 32:base + 64] = dk[None, :]
    return (cq.astype(BF16), sq.astype(BF16), ck.astype(BF16), sk.astype(BF16))


def _pt_matrix():
    """lhsT of the rotate-half block-swap matrix (out = P @ rhs)."""
    Pm = np.zeros((P, P), np.float32)
    for b0 in (0, 64):
        for rr in range(16):
            Pm[b0 + rr, b0 + 16 + rr] = -1.0
            Pm[b0 + 16 + rr, b0 + rr] = 1.0
    return Pm.T.astype(BF16)  # Pt[k, m] = P[m, k]


def _tri_mask():
    tri = (np.arange(P)[None, :] >= np.arange(P)[:, None])
    return tri.astype(BF16)  # tri[rj, t] = t >= rj


def _blockones():
    k = np.arange(P)
    return (k[:, None] // 64 == k[None, :] // 64).astype(BF16)


def _rep2(vec_slice):
    """[256] channel vector -> [128, 2] f32 (per-partition, per m-tile)."""
    out = np.empty((P, 2), np.float32)
    for mt in range(2):
        out[:, mt] = vec_slice[mt * P:(mt + 1) * P]
    return out


def _host_prep(inputs):
    x = np.asarray(inputs["x"], np.float32)
    Wq = np.asarray(inputs["Wq"], np.float32)
    Wk = np.asarray(inputs["Wk"], np.float32)
    Wv = np.asarray(inputs["Wv"], np.float32)
    Wg = np.asarray(inputs["Wg"], np.float32)
    Wo = np.asarray(inputs["Wo"], np.float32)
    bq = np.asarray(inputs["bq"], np.float32)
    bk = np.asarray(inputs["bk"], np.float32)
    bv = np.asarray(inputs["bv"], np.float32)
    bg = np.asarray(inputs["bg"], np.float32)
    gn_w = np.asarray(inputs["gn_w"], np.float32)
    gn_b = np.asarray(inputs["gn_b"], np.float32)

    pt = _pt_matrix()
    tri = _tri_mask()
    ob = _blockones()
    has_bv = bool(np.any(bv != 0.0))

    in_maps = []
    for core in range(N_CORES):
        b = core // HG
        hg = core % HG
        h0 = HPC * hg
        cols = slice(NCH * hg, NCH * (hg + 1))
        cq, sq, ck, sk = _cs_tiles(h0)
        gnw_rep = np.empty((P, 2), np.float32)
        gnb_rep = np.empty((P, 2), np.float32)
        for mt in range(2):
            for half in range(2):
                g = h0 + 2 * mt + half
                gnw_rep[64 * half:64 * (half + 1), mt] = gn_w[g]
                gnb_rep[64 * half:64 * (half + 1), mt] = gn_b[g]
        m = {
            "xt": np.ascontiguousarray(x[b].T).astype(BF16),
            "wq": _perm_cols(Wq[:, cols]).astype(BF16),
            "wk": _perm_cols(Wk[:, cols]).astype(BF16),
            "wv": np.ascontiguousarray(Wv[:, cols]).astype(BF16),
            "wg": np.ascontiguousarray(Wg[:, cols]).astype(BF16),
            "wo": np.ascontiguousarray(Wo[cols, :]).astype(BF16),
            "cq": cq, "sq": sq, "ck": ck, "sk": sk,
            "pt": pt, "tri": tri, "ob": ob,
            "gnw": gnw_rep, "gnb": gnb_rep,
            "bqr": _rep2(_perm_cols(bq[None, cols])[0]),
            "bkr": _rep2(_perm_cols(bk[None, cols])[0]),
            "bgr": _rep2(bg[cols]),
        }
        if has_bv:
            m["bvb"] = np.broadcast_to(bv[cols][None, :], (P, NCH)).astype(
                np.float32).copy()
        in_maps.append(m)
    return in_maps, has_bv


class _CopyBalancer:
    """Greedy ACT/DVE load balancer for PSUM->SBUF copies."""

    def __init__(self, nc, act_seed_ns, dve_seed_ns):
        self.nc = nc
        self.act = float(act_seed_ns)
        self.dve = float(dve_seed_ns)

    def copy(self, dst, src):
        fd = src.free_size()
        act_cost = (352.0 + fd) / 1.2
        dve_cost = (120.0 + fd) / 0.96
        if self.act + act_cost <= self.dve + dve_cost:
            self.act += act_cost
            self.nc.scalar.copy(dst, src)
        else:
            self.dve += dve_cost
            self.nc.vector.tensor_copy(dst, src)


def _build_program(has_bv, sim_safe=False):
    nc = bacc.Bacc("TRN2", target_bir_lowering=False, debug=False,
                   num_devices=N_CORES)
    f32 = mybir.dt.float32
    bf16 = mybir.dt.bfloat16

    xt_d = nc.dram_tensor("xt", [D, S], bf16, kind="ExternalInput")
    wq_d = nc.dram_tensor("wq", [D, NCH], bf16, kind="ExternalInput")
    wk_d = nc.dram_tensor("wk", [D, NCH], bf16, kind="ExternalInput")
    wv_d = nc.dram_tensor("wv", [D, NCH], bf16, kind="ExternalInput")
    wg_d = nc.dram_tensor("wg", [D, NCH], bf16, kind="ExternalInput")
    wo_d = nc.dram_tensor("wo", [NCH, D], bf16, kind="ExternalInput")
    cq_d = nc.dram_tensor("cq", [2, P, S], bf16, kind="ExternalInput")
    sq_d = nc.dram_tensor("sq", [2, P, S], bf16, kind="ExternalInput")
    ck_d = nc.dram_tensor("ck", [2, P, S], bf16, kind="ExternalInput")
    sk_d = nc.dram_tensor("sk", [2, P, S], bf16, kind="ExternalInput")
    pt_d = nc.dram_tensor("pt", [P, P], bf16, kind="ExternalInput")
    tri_d = nc.dram_tensor("tri", [P, P], bf16, kind="ExternalInput")
    ob_d = nc.dram_tensor("ob", [P, P], bf16, kind="ExternalInput")
    gnw_d = nc.dram_tensor("gnw", [P, 2], f32, kind="ExternalInput")
    gnb_d = nc.dram_tensor("gnb", [P, 2], f32, kind="ExternalInput")
    bqr_d = nc.dram_tensor("bqr", [P, 2], f32, kind="ExternalInput")
    bkr_d = nc.dram_tensor("bkr", [P, 2], f32, kind="ExternalInput")
    bgr_d = nc.dram_tensor("bgr", [P, 2], f32, kind="ExternalInput")
    bvb_d = (nc.dram_tensor("bvb", [P, NCH], f32, kind="ExternalInput")
             if has_bv else None)
    out_d = nc.dram_tensor("out", [S, D], f32, kind="ExternalOutput")

    ident = mybir.ActivationFunctionType.Identity
    silu = mybir.ActivationFunctionType.Silu
    fcopy = mybir.ActivationFunctionType.Copy
    fsquare = mybir.ActivationFunctionType.Square
    fsqrt = mybir.ActivationFunctionType.Sqrt
    mul_op = mybir.AluOpType.mult
    add_op = mybir.AluOpType.add
    sub_op = mybir.AluOpType.subtract

    with tile.TileContext(nc) as tc:
        with (
            tc.tile_pool(name="consts", bufs=1) as cpool,
            tc.tile_pool(name="wts", bufs=1) as wpool,
            tc.tile_pool(name="big", bufs=1) as big,
            tc.tile_pool(name="ps", bufs=4, space="PSUM") as ps,
            tc.tile_pool(name="psacc", bufs=2, space="PSUM") as psacc,
            tc.tile_pool(name="pssm", bufs=2, space="PSUM") as pssm,
            tc.tile_pool(name="scp", bufs=6) as scp,
            tc.tile_pool(name="rotp", bufs=6) as rotp,
            tc.tile_pool(name="nrmp", bufs=2) as nrmp,
            tc.tile_pool(name="outp", bufs=4) as outp,
            tc.tile_pool(name="finp", bufs=1) as finp,
        ):
            # ---- load constants / inputs ----
            xtT = big.tile([P, 8, S], bf16)
            nc.sync.dma_start(xtT[:, :, :],
                              xt_d.ap().rearrange("(c p) s -> p c s", p=P))
            wT = {}
            for nm, dh in (("wq", wq_d), ("wk", wk_d), ("wv", wv_d),
                           ("wg", wg_d)):
                t = wpool.tile([P, 8, NCH], bf16, tag=nm)
                nc.sync.dma_start(t[:, :, :],
                                  dh.ap().rearrange("(c p) n -> p c n", p=P))
                wT[nm] = t
            woT = wpool.tile([P, 2, D], bf16, tag="wo")
            nc.sync.dma_start(woT[:, :, :],
                              wo_d.ap().rearrange("(c p) n -> p c n", p=P))
            csT = {}
            for nm, dh in (("cq", cq_d), ("sq", sq_d), ("ck", ck_d),
                           ("sk", sk_d)):
                t = cpool.tile([P, 2, S], bf16, tag=nm)
                nc.sync.dma_start(t[:, :, :], dh.ap().rearrange("i p s -> p i s"))
                csT[nm] = t
            ptT = cpool.tile([P, P], bf16, tag="pt")
            nc.sync.dma_start(ptT[:, :], pt_d[:, :])
            triT = cpool.tile([P, P], bf16, tag="tri")
            nc.sync.dma_start(triT[:, :], tri_d[:, :])
            obT = cpool.tile([P, P], bf16, tag="ob")
            nc.sync.dma_start(obT[:, :], ob_d[:, :])
            gnwT = cpool.tile([P, 2], f32, tag="gnw")
            nc.sync.dma_start(gnwT[:, :], gnw_d[:, :])
            gnbT = cpool.tile([P, 2], f32, tag="gnb")
            nc.sync.dma_start(gnbT[:, :], gnb_d[:, :])
            biasT = {}
            for nm, dh in (("bqr", bqr_d), ("bkr", bkr_d), ("bgr", bgr_d)):
                t = cpool.tile([P, 2], f32, tag=nm)
                nc.sync.dma_start(t[:, :], dh[:, :])
                biasT[nm] = t
            zeroT = cpool.tile([P, 1], f32, tag="zero")
            nc.vector.memset(zeroT[:, :], 0.0)
            epsT = cpool.tile([P, 1], f32, tag="eps")
            nc.vector.memset(epsT[:, :], GN_EPS)
            if has_bv:
                bvbT = cpool.tile([P, NCH], f32, tag="bvb")
                nc.sync.dma_start(bvbT[:, :], bvb_d[:, :])

            qhT = big.tile([P, 2, S], bf16, tag="qh")
            khT = big.tile([P, 2, S], bf16, tag="kh")
            # v stored zero-padded per head: [K, hl] -> [128, 128] with the
            # head's 64 columns placed at rows 0:64 (even hl) or 64:128 (odd
            # hl) of the matmul output, so every V matmul writes the full
            # 128-partition PSUM region at offset 0.
            vT = big.tile([P, NKB, HPC, P], bf16, tag="v")
            nc.gpsimd.memset(vT[:, :, :, :], 0.0)
            gateT = big.tile([P, 2, S], bf16, tag="gate")
            retT = big.tile([P, 2, S], bf16, tag="ret")
            gtdT = big.tile([P, 2, S], bf16, tag="gtd")
            statT = finp.tile([P, 16], f32, tag="stat")

            # seeds: fixed ACT work ~29us, fixed DVE work ~48us (see notes)
            bal = _CopyBalancer(nc, 29000.0, 48000.0)

            # ---- Phase A: projections ----
            for nm, dst, cnm, snm, bnm in (("wq", qhT, "cq", "sq", "bqr"),
                                           ("wk", khT, "ck", "sk", "bkr")):
                for mt in range(2):
                    for c in range(NQC):
                        pst = ps.tile([P, QCH], f32, tag="mm")
                        for kc in range(8):
                            nc.tensor.matmul(
                                pst[:, :],
                                lhsT=wT[nm][:, kc, mt * P:(mt + 1) * P],
                                rhs=xtT[:, kc, c * QCH:(c + 1) * QCH],
                                start=(kc == 0), stop=(kc == 7))
                        qc = rotp.tile([P, QCH], bf16, tag="rt")
                        nc.scalar.activation(qc[:, :], pst[:, :], ident,
                                             bias=biasT[bnm][:, mt:mt + 1],
                                             scale=1.0)
                        csl = slice(c * QCH, (c + 1) * QCH)
                        t1 = rotp.tile([P, QCH], bf16, tag="rt")
                        nc.vector.tensor_tensor(t1[:, :], qc[:, :],
                                                csT[cnm][:, mt, csl], mul_op)
                        t2 = rotp.tile([P, QCH], bf16, tag="rt")
                        nc.vector.tensor_tensor(t2[:, :], qc[:, :],
                                                csT[snm][:, mt, csl], mul_op)
                        psw = ps.tile([P, QCH], f32, tag="mm")
                        nc.tensor.matmul(psw[:, :], lhsT=ptT[:, :], rhs=t2[:, :],
                                         start=True, stop=True)
                        nc.vector.tensor_tensor(dst[:, mt, csl], t1[:, :],
                                                psw[:, :], add_op)
            # gate projection (transposed) + silu
            for mt in range(2):
                for c in range(NQC):
                    pst = ps.tile([P, QCH], f32, tag="mm")
                    for kc in range(8):
                        nc.tensor.matmul(
                            pst[:, :],
                            lhsT=wT["wg"][:, kc, mt * P:(mt + 1) * P],
                            rhs=xtT[:, kc, c * QCH:(c + 1) * QCH],
                            start=(kc == 0), stop=(kc == 7))
                    if sim_safe:
                        # CoreSim lacks Silu; emulate as sigmoid(x)*x
                        # (valid only for zero bg, which holds in practice)
                        sgt = rotp.tile([P, QCH], bf16, tag="rt")
                        nc.scalar.activation(
                            sgt[:, :], pst[:, :],
                            mybir.ActivationFunctionType.Sigmoid,
                            bias=biasT["bgr"][:, mt:mt + 1], scale=1.0)
                        nc.vector.tensor_tensor(
                            gateT[:, mt, c * QCH:(c + 1) * QCH],
                            sgt[:, :], pst[:, :], mul_op)
                    else:
                        nc.scalar.activation(
                            gateT[:, mt, c * QCH:(c + 1) * QCH],
                            pst[:, :], silu,
                            bias=biasT["bgr"][:, mt:mt + 1], scale=1.0)
            # v projection (natural layout)
            for tt in range(NKB):
                pst = ps.tile([P, QCH], f32, tag="mm")
                for kc in range(8):
                    nc.tensor.matmul(
                        pst[:, :NCH],
                        lhsT=xtT[:, kc, tt * P:(tt + 1) * P],
                        rhs=wT["wv"][:, kc, :],
                        start=(kc == 0), stop=(kc == 7))
                for hl in range(HPC):
                    dstv = vT[:, tt, hl, (hl % 2) * DH:(hl % 2) * DH + DH]
                    srcv = pst[:, hl * DH:(hl + 1) * DH]
                    if has_bv:
                        nc.vector.tensor_tensor(dstv, srcv,
                                                bvbT[:, hl * DH:(hl + 1) * DH],
                                                add_op)
                    else:
                        nc.vector.tensor_copy(dstv, srcv)

            # ---- Phase B: retention (scores + V) ----
            for mt in range(2):
                for c in range(NQC):
                    rp = psacc.tile([P, QCH], f32, tag="acc")
                    for half in range(2):
                        prow = 64 * half
                        hl = 2 * mt + half
                        nk = 4 * c + 4
                        for K in range(nk):
                            o = K - 4 * c
                            qsl = slice(c * QCH, (c + 1) * QCH)
                            ksl = slice(K * KBLK, (K + 1) * KBLK)
                            lk = khT[prow:prow + 64, mt, ksl]
                            vstart = (half == 0 and K == 0)
                            vstop = (half == 1 and K == nk - 1)
                            if o < 0:
                                sp = ps.tile([P, QCH], f32, tag="mm")
                                nc.tensor.matmul(
                                    sp[:, :], lhsT=lk,
                                    rhs=qhT[prow:prow + 64, mt, qsl],
                                    start=True, stop=True)
                                ss = scp.tile([P, QCH], bf16, tag="sc")
                                bal.copy(ss[:, :], sp[:, :])
                                nc.tensor.matmul(
                                    rp[:, :],
                                    lhsT=vT[:, K, hl, :],
                                    rhs=ss[:, :],
                                    start=vstart, stop=vstop)
                            else:
                                w = QCH - KBLK * o
                                sp = ps.tile([P, QCH], f32, tag="mm")
                                nc.tensor.matmul(
                                    sp[:, :w], lhsT=lk,
                                    rhs=qhT[prow:prow + 64, mt,
                                            c * QCH + KBLK * o:(c + 1) * QCH],
                                    start=True, stop=True)
                                ss = scp.tile([P, QCH], bf16, tag="sc")
                                nc.vector.tensor_tensor(ss[:, :KBLK],
                                                        sp[:, :KBLK],
                                                        triT[:, :], mul_op)
                                bal.dve += (120.0 + KBLK) / 0.96
                                if o < 3:
                                    bal.copy(ss[:, KBLK:w], sp[:, KBLK:w])
                                nc.tensor.matmul(
                                    rp[:, KBLK * o:QCH],
                                    lhsT=vT[:, K, hl, :],
                                    rhs=ss[:, :w],
                                    start=vstart, stop=vstop)
                    # stats + spill to SBUF (both heads at once)
                    sidx = mt * NQC + c
                    nc.scalar.activation(retT[:, mt, c * QCH:(c + 1) * QCH],
                                         rp[:, :], fcopy,
                                         accum_out=statT[:, sidx:sidx + 1])
                    sqs = scp.tile([P, QCH], bf16, tag="sc")
                    nc.scalar.activation(sqs[:, :], rp[:, :], fsquare,
                                         bias=zeroT[:, :],
                                         accum_out=statT[:, 8 + sidx:9 + sidx])

            # ---- Phase C: groupnorm finalize + gate + output projection ----
            s1 = finp.tile([P, 4], f32, tag="s1")
            nc.vector.tensor_reduce(
                s1[:, :], statT[:, :].rearrange("p (g c) -> p g c", c=NQC),
                axis=mybir.AxisListType.X, op=add_op)
            s1b = finp.tile([P, 4], bf16, tag="s1b")
            nc.vector.tensor_copy(s1b[:, :], s1[:, :])
            totp = pssm.tile([P, 4], f32, tag="tot")
            nc.tensor.matmul(totp[:, :], lhsT=obT[:, :], rhs=s1b[:, :],
                             start=True, stop=True)
            tot = finp.tile([P, 4], f32, tag="tot_sb")
            nc.vector.tensor_copy(tot[:, :], totp[:, :])
            mean = finp.tile([P, 2], f32, tag="mean")
            nc.vector.tensor_scalar_mul(mean[:, :], tot[:, 0:2], 1.0 / NELEM)
            ex2 = finp.tile([P, 2], f32, tag="ex2")
            nc.vector.tensor_scalar_mul(ex2[:, :], tot[:, 2:4], 1.0 / NELEM)
            msq = finp.tile([P, 2], f32, tag="msq")
            nc.vector.tensor_tensor(msq[:, :], mean[:, :], mean[:, :], mul_op)
            var = finp.tile([P, 2], f32, tag="var")
            nc.vector.tensor_tensor(var[:, :], ex2[:, :], msq[:, :], sub_op)
            std = finp.tile([P, 2], f32, tag="std")
            nc.scalar.activation(std[:, :], var[:, :], fsqrt,
                                 bias=epsT[:, :], scale=1.0)
            istd = finp.tile([P, 2], f32, tag="istd")
            nc.vector.reciprocal(istd[:, :], std[:, :])
            aff_a = finp.tile([P, 2], f32, tag="aff_a")
            nc.vector.tensor_tensor(aff_a[:, :], istd[:, :], gnwT[:, :], mul_op)
            ma = finp.tile([P, 2], f32, tag="ma")
            nc.vector.tensor_tensor(ma[:, :], mean[:, :], aff_a[:, :], mul_op)
            aff_b = finp.tile([P, 2], f32, tag="aff_b")
            nc.vector.tensor_tensor(aff_b[:, :], gnbT[:, :], ma[:, :], sub_op)

            for mt in range(2):
                nrm = nrmp.tile([P, S], bf16, tag="nrm")
                nc.vector.tensor_scalar(nrm[:, :], retT[:, mt, :],
                                        aff_a[:, mt:mt + 1],
                                        aff_b[:, mt:mt + 1], mul_op, add_op)
                nc.vector.tensor_tensor(gtdT[:, mt, :], nrm[:, :],
                                        gateT[:, mt, :], mul_op)

            for tt in range(NKB):
                for oc in range(2):
                    op_ps = psacc.tile([P, QCH], f32, tag="acc")
                    for kc in range(2):
                        nc.tensor.matmul(
                            op_ps[:, :],
                            lhsT=gtdT[:, kc, tt * P:(tt + 1) * P],
                            rhs=woT[:, kc, oc * QCH:(oc + 1) * QCH],
                            start=(kc == 0), stop=(kc == 1))
                    ob_t = outp.tile([P, QCH], f32, tag="ob")
                    bal.copy(ob_t[:, :], op_ps[:, :])
                    nc.sync.dma_start(
                        out_d[tt * P:(tt + 1) * P, oc * QCH:(oc + 1) * QCH],
                        ob_t[:, :])

    nc.compile()
    return nc


_PROGRAM_CACHE = {}


def _get_program(has_bv):
    if has_bv not in _PROGRAM_CACHE:
        _PROGRAM_CACHE[has_bv] = _build_program(has_bv)
    return _PROGRAM_CACHE[has_bv]


def kernel(**inputs):
    global LAST_EXEC_NS, LAST_RESULTS
    in_maps, has_bv = _host_prep(inputs)
    nc = _get_program(has_bv)
    trace = bool(int(os.environ.get("KERNEL_TRACE", "0")))
    kw = {}
    if trace:
        kw["trace"] = True
        kw["trace_cores"] = [int(c) for c in
                             os.environ.get("KERNEL_TRACE_CORES", "0").split(",")]
        td = os.environ.get("KERNEL_TRACE_DIR")
        if td:
            os.makedirs(td, exist_ok=True)
            kw["tmpdir"] = td
    res = run_bass_kernel_spmd(nc, in_maps, list(range(N_CORES)), **kw)
    LAST_EXEC_NS = res.exec_time_ns
    LAST_RESULTS = res
    bo = np.asarray(inputs["bo"], np.float32)
    out = np.zeros((B, S, D), np.float32)
    for core in range(N_CORES):
        out[core // HG] += res.results[core]["out"]
    out += bo[None, None, :]
    return out



# revision 69
# speedup vs baseline: 2.4581x; 2.4581x over previous
"""Trainium2 Bass kernel for GatedMultiScaleRetentionLayer.

Sharding: 8 cores = data-parallel over batch (B=2) x tensor-parallel over
heads (16 heads -> 4 groups of 4). Each core computes its batch's tokens for
its 4 heads end-to-end; host sums the 4 bf16 partial outputs per batch and
adds bo.

Retention uses the chunked-state form instead of materializing all causal
score blocks: with decay folded per token (qhat_i = rot(q_i) gamma^i rownorm_i,
khat_j = rot(k_j) gamma^-j), the running state S = sum_j khat_j v_j^T
(accumulated in PSUM across 128-token chunks) turns all cross-chunk history
into one [64,64] matmul per head per chunk. Only the intra-chunk 128x128
block needs explicit scores + triangular masking. This cuts retention PE
work ~4x vs the all-blocks form.

The interleaved rotary is made partition-friendly by permuting the rotary
dims of Wq/Wk columns (even indices first), so rotate_half becomes a fixed
16-row block swap done with one small PE matmul against a +-1 permutation
matrix. khat in token-major layout (for the state update) is produced by
SBUF->SBUF DMA XBAR transposes.
"""

import os

import numpy as np
import ml_dtypes

import concourse.bass as bass
import concourse.tile as tile
from concourse import bacc, mybir
from concourse.bass_utils import run_bass_kernel_spmd

BF16 = ml_dtypes.bfloat16

# ---- problem constants (hardcoded per contract) ----
B = 2
S = 2048
D = 1024
H = 16
DH = 64
ROT = 32
THETA = 10000.0
XPOS_BASE = 512.0
GN_EPS = 1e-5

N_CORES = 8
HG = 4          # head groups (tensor-parallel)
HPC = 4         # heads per core
NCH = HPC * DH  # 256 channels per core
P = 128
QCH = 512       # 512-token projection chunk (one PSUM bank of fp32)
NQC = S // QCH  # 4 projection chunks per mt
CK = 128        # retention chunk
NC = S // CK    # 16 retention chunks
NELEM = float(S * DH)  # groupnorm element count per head

LAST_EXEC_NS = None
LAST_RESULTS = None

_PERM = np.concatenate([np.arange(0, ROT, 2), np.arange(1, ROT, 2),
                        np.arange(ROT, DH)])  # within-head column permutation


def _perm_cols(w_slice):
    """Permute rotary dims of each head's 64-column block (even idx first)."""
    out = np.empty_like(w_slice)
    for hl in range(HPC):
        blk = w_slice[..., hl * DH:(hl + 1) * DH]
        out[..., hl * DH:(hl + 1) * DH] = blk[..., _PERM]
    return out


def _rot_tables():
    """angle[t, r], xpos scale[t, r] for pair index r in [0,16)."""
    t = np.arange(S, dtype=np.float64)
    r = np.arange(ROT // 2, dtype=np.float64)
    inv_freq = 1.0 / (THETA ** ((2.0 * r) / ROT))
    angle = t[:, None] * inv_freq[None, :]                   # [S, 16]
    base = (2.0 * r + 0.4 * ROT) / (1.4 * ROT)               # [16]
    power = (t - S // 2) / XPOS_BASE                         # [S]
    scale = base[None, :] ** power[:, None]                  # [S, 16]
    return angle, scale


def _decay_factors():
    """gamma^i*rownorm (for q) and gamma^-j (for k), per global head. f64."""
    h = np.arange(H, dtype=np.float64)
    gamma = 1.0 - 2.0 ** (-5.0 - h)                          # [H]
    t = np.arange(S, dtype=np.float64)
    logg = np.log(gamma)
    g_pos = np.exp(t[None, :] * logg[:, None])               # [H, S] gamma^t
    g_neg = np.exp(-t[None, :] * logg[:, None])              # [H, S] gamma^-t
    rowsum = (1.0 - gamma[:, None] * g_pos) / (1.0 - gamma[:, None])
    rownorm = 1.0 / np.sqrt(rowsum)                          # [H, S]
    return g_pos * rownorm, g_neg


def _cs_tiles(h0):
    """cq, sq, ck, sk tiles [2, 128, S] bf16 for heads h0..h0+3."""
    angle, scale = _rot_tables()
    dq_all, dk_all = _decay_factors()
    cos, sin = np.cos(angle), np.sin(angle)                  # [S, 16]
    cq = np.zeros((2, P, S), np.float64)
    sq = np.zeros((2, P, S), np.float64)
    ck = np.zeros((2, P, S), np.float64)
    sk = np.zeros((2, P, S), np.float64)
    for mt in range(2):
        for half in range(2):
            hl = 2 * mt + half
            g = h0 + hl
            dq = dq_all[g]                                   # [S]
            dk = dk_all[g]
            base = 64 * half
            for rr in range(16):
                cq[mt, base + rr] = cos[:, rr] * scale[:, rr] * dq
                cq[mt, base + 16 + rr] = cos[:, rr] * scale[:, rr] * dq
                sq[mt, base + rr] = sin[:, rr] * scale[:, rr] * dq
                sq[mt, base + 16 + rr] = sin[:, rr] * scale[:, rr] * dq
                ck[mt, base + rr] = cos[:, rr] / scale[:, rr] * dk
                ck[mt, base + 16 + rr] = cos[:, rr] / scale[:, rr] * dk
                sk[mt, base + rr] = sin[:, rr] / scale[:, rr] * dk
                sk[mt, base + 16 + rr] = sin[:, rr] / scale[:, rr] * dk
            cq[mt, base + 32:base + 64] = dq[None, :]
            ck[mt, base + 32:base + 64] = dk[None, :]
    return (cq.astype(BF16), sq.astype(BF16), ck.astype(BF16), sk.astype(BF16))


def _pt_matrix():
    """lhsT of the rotate-half block-swap matrix (out = P @ rhs)."""
    Pm = np.zeros((P, P), np.float32)
    for b0 in (0, 64):
        for rr in range(16):
            Pm[b0 + rr, b0 + 16 + rr] = -1.0
            Pm[b0 + 16 + rr, b0 + rr] = 1.0
    return Pm.T.astype(BF16)  # Pt[k, m] = P[m, k]


def _tri_mask():
    tri = (np.arange(P)[None, :] >= np.arange(P)[:, None])
    return np.tile(tri, (1, 4)).astype(BF16)  # tri[rj, t%128] = t >= rj, x4


def _blockones():
    k = np.arange(P)
    return (k[:, None] // 64 == k[None, :] // 64).astype(BF16)


def _rep2(vec_slice):
    """[256] channel vector -> [128, 2] f32 (per-partition, per m-tile)."""
    out = np.empty((P, 2), np.float32)
    for mt in range(2):
        out[:, mt] = vec_slice[mt * P:(mt + 1) * P]
    return out


def _host_prep(inputs):
    x = np.asarray(inputs["x"], np.float32)
    Wq = np.asarray(inputs["Wq"], np.float32)
    Wk = np.asarray(inputs["Wk"], np.float32)
    Wv = np.asarray(inputs["Wv"], np.float32)
    Wg = np.asarray(inputs["Wg"], np.float32)
    Wo = np.asarray(inputs["Wo"], np.float32)
    bq = np.asarray(inputs["bq"], np.float32)
    bk = np.asarray(inputs["bk"], np.float32)
    bv = np.asarray(inputs["bv"], np.float32)
    bg = np.asarray(inputs["bg"], np.float32)
    gn_w = np.asarray(inputs["gn_w"], np.float32)
    gn_b = np.asarray(inputs["gn_b"], np.float32)

    pt = _pt_matrix()
    tri = _tri_mask()
    ob = _blockones()
    has_bv = bool(np.any(bv != 0.0))

    in_maps = []
    for core in range(N_CORES):
        b = core // HG
        hg = core % HG
        h0 = HPC * hg
        cols = slice(NCH * hg, NCH * (hg + 1))
        cq, sq, ck, sk = _cs_tiles(h0)
        gnw_rep = np.empty((P, 2), np.float32)
        gnb_rep = np.empty((P, 2), np.float32)
        for mt in range(2):
            for half in range(2):
                g = h0 + 2 * mt + half
                gnw_rep[64 * half:64 * (half + 1), mt] = gn_w[g]
                gnb_rep[64 * half:64 * (half + 1), mt] = gn_b[g]
        def pmaj_w(w, nb=8):
            # [nb*P, N] -> [P, nb, N] so each partition's data is contiguous
            return np.ascontiguousarray(
                w.reshape(nb, P, -1).transpose(1, 0, 2)).astype(BF16)

        xt = x[b].T.reshape(8, P, S).transpose(1, 0, 2)      # [P, 8, S]
        m = {
            "wq": pmaj_w(_perm_cols(Wq[:, cols])),
            "wk": pmaj_w(_perm_cols(Wk[:, cols])),
            "wv": pmaj_w(Wv[:, cols]),
            "wg": pmaj_w(Wg[:, cols]),
            "wo": pmaj_w(Wo[cols, :], nb=2),
            "cq": np.ascontiguousarray(cq.transpose(1, 0, 2)),
            "sq": np.ascontiguousarray(sq.transpose(1, 0, 2)),
            "ck": np.ascontiguousarray(ck.transpose(1, 0, 2)),
            "sk": np.ascontiguousarray(sk.transpose(1, 0, 2)),
            "pt": pt, "tri": tri, "ob": ob,
            "gnw": gnw_rep, "gnb": gnb_rep,
            "bqr": _rep2(_perm_cols(bq[None, cols])[0]),
            "bkr": _rep2(_perm_cols(bk[None, cols])[0]),
            "bgr": _rep2(bg[cols]),
        }
        for cc4 in range(4):
            m[f"xt{cc4}"] = np.ascontiguousarray(
                xt[:, :, cc4 * QCH:(cc4 + 1) * QCH]).astype(BF16)
        if has_bv:
            m["bvb"] = np.broadcast_to(bv[cols][None, :], (P, NCH)).astype(
                np.float32).copy()
        in_maps.append(m)
    return in_maps, has_bv


def _build_program(has_bv):
    nc = bacc.Bacc("TRN2", target_bir_lowering=False, debug=False,
                   num_devices=N_CORES)
    f32 = mybir.dt.float32
    bf16 = mybir.dt.bfloat16

    xt_ds = [nc.dram_tensor(f"xt{c}", [P, 8, QCH], bf16, kind="ExternalInput")
             for c in range(4)]
    wq_d = nc.dram_tensor("wq", [P, 8, NCH], bf16, kind="ExternalInput")
    wk_d = nc.dram_tensor("wk", [P, 8, NCH], bf16, kind="ExternalInput")
    wv_d = nc.dram_tensor("wv", [P, 8, NCH], bf16, kind="ExternalInput")
    wg_d = nc.dram_tensor("wg", [P, 8, NCH], bf16, kind="ExternalInput")
    wo_d = nc.dram_tensor("wo", [P, 2, D], bf16, kind="ExternalInput")
    cq_d = nc.dram_tensor("cq", [P, 2, S], bf16, kind="ExternalInput")
    sq_d = nc.dram_tensor("sq", [P, 2, S], bf16, kind="ExternalInput")
    ck_d = nc.dram_tensor("ck", [P, 2, S], bf16, kind="ExternalInput")
    sk_d = nc.dram_tensor("sk", [P, 2, S], bf16, kind="ExternalInput")
    pt_d = nc.dram_tensor("pt", [P, P], bf16, kind="ExternalInput")
    tri_d = nc.dram_tensor("tri", [P, 4 * P], bf16, kind="ExternalInput")
    ob_d = nc.dram_tensor("ob", [P, P], bf16, kind="ExternalInput")
    gnw_d = nc.dram_tensor("gnw", [P, 2], f32, kind="ExternalInput")
    gnb_d = nc.dram_tensor("gnb", [P, 2], f32, kind="ExternalInput")
    bqr_d = nc.dram_tensor("bqr", [P, 2], f32, kind="ExternalInput")
    bkr_d = nc.dram_tensor("bkr", [P, 2], f32, kind="ExternalInput")
    bgr_d = nc.dram_tensor("bgr", [P, 2], f32, kind="ExternalInput")
    bvb_d = (nc.dram_tensor("bvb", [P, NCH], f32, kind="ExternalInput")
             if has_bv else None)
    out_d = nc.dram_tensor("out", [S, D], bf16, kind="ExternalOutput")

    ident = mybir.ActivationFunctionType.Identity
    silu = mybir.ActivationFunctionType.Silu
    fcopy = mybir.ActivationFunctionType.Copy
    fsquare = mybir.ActivationFunctionType.Square
    fsqrt = mybir.ActivationFunctionType.Sqrt
    mul_op = mybir.AluOpType.mult
    add_op = mybir.AluOpType.add
    sub_op = mybir.AluOpType.subtract

    with tile.TileContext(nc) as tc:
        with (
            tc.tile_pool(name="consts", bufs=1) as cpool,
            tc.tile_pool(name="wts", bufs=1) as wpool,
            tc.tile_pool(name="big", bufs=1) as big,
            tc.tile_pool(name="mm", bufs=4, space="PSUM") as psmm,
            tc.tile_pool(name="acc", bufs=3, space="PSUM") as psacc,
            tc.tile_pool(name="state", bufs=1, space="PSUM") as psst,
            tc.tile_pool(name="rotp", bufs=9) as rotp,
            tc.tile_pool(name="ssp", bufs=10) as ssp,
            tc.tile_pool(name="sbp", bufs=2) as sbp,
            tc.tile_pool(name="sqp", bufs=2) as sqp,
            tc.tile_pool(name="nrmp", bufs=4) as nrmp,
            tc.tile_pool(name="outp", bufs=4) as outp,
            tc.tile_pool(name="finp", bufs=1) as finp,
        ):
            # ---- input DMAs: all host-prelayouted to partition-major so
            # every descriptor is one contiguous row per partition (cheap to
            # generate), ordered critical-first ----
            wT = {}
            for nm in ("wq", "wk", "wv", "wg"):
                wT[nm] = wpool.tile([P, 8, NCH], bf16, tag=nm, name=nm)
            xtT = big.tile([P, 4, 8, QCH], bf16, tag="xt")
            csT = {}
            for nm in ("cq", "sq", "ck", "sk"):
                csT[nm] = cpool.tile([P, 2, S], bf16, tag=nm, name=nm)

            nc.sync.dma_start(wT["wk"][:, :, :], wk_d[:, :, :])
            nc.sync.dma_start(xtT[:, 0, :, :], xt_ds[0][:, :, :])
            biasT = {}
            for nm, dh in (("bqr", bqr_d), ("bkr", bkr_d)):
                t = cpool.tile([P, 2], f32, tag=nm, name=nm)
                nc.sync.dma_start(t[:, :], dh[:, :])
                biasT[nm] = t
            def cs_chunk(nm, dh, c):
                sl = slice(c * QCH, (c + 1) * QCH)
                nc.sync.dma_start(csT[nm][:, :, sl], dh[:, :, sl])

            cs_chunk("ck", ck_d, 0)
            cs_chunk("sk", sk_d, 0)
            ptT = cpool.tile([P, P], bf16, tag="pt")
            nc.sync.dma_start(ptT[:, :], pt_d[:, :])
            nc.sync.dma_start(xtT[:, 1, :, :], xt_ds[1][:, :, :])
            cs_chunk("ck", ck_d, 1)
            cs_chunk("sk", sk_d, 1)
            nc.sync.dma_start(xtT[:, 2, :, :], xt_ds[2][:, :, :])
            cs_chunk("ck", ck_d, 2)
            cs_chunk("sk", sk_d, 2)
            nc.sync.dma_start(wT["wq"][:, :, :], wq_d[:, :, :])
            nc.sync.dma_start(xtT[:, 3, :, :], xt_ds[3][:, :, :])
            cs_chunk("ck", ck_d, 3)
            cs_chunk("sk", sk_d, 3)
            for c in range(NQC):
                cs_chunk("cq", cq_d, c)
                cs_chunk("sq", sq_d, c)
            triT = cpool.tile([P, 4 * P], bf16, tag="tri")
            nc.sync.dma_start(triT[:, :], tri_d[:, :])
            nc.sync.dma_start(wT["wv"][:, :, :], wv_d[:, :, :])
            nc.sync.dma_start(wT["wg"][:, :, :], wg_d[:, :, :])
            woT = wpool.tile([P, 2, D], bf16, tag="wo")
            nc.sync.dma_start(woT[:, :, :], wo_d[:, :, :])
            obT = cpool.tile([P, P], bf16, tag="ob")
            nc.sync.dma_start(obT[:, :], ob_d[:, :])
            gnwT = cpool.tile([P, 2], f32, tag="gnw")
            nc.sync.dma_start(gnwT[:, :], gnw_d[:, :])
            gnbT = cpool.tile([P, 2], f32, tag="gnb")
            nc.sync.dma_start(gnbT[:, :], gnb_d[:, :])
            t = cpool.tile([P, 2], f32, tag="bgr", name="bgr")
            nc.sync.dma_start(t[:, :], bgr_d[:, :])
            biasT["bgr"] = t
            zeroT = cpool.tile([P, 1], f32, tag="zero")
            nc.vector.memset(zeroT[:, :], 0.0)
            epsT = cpool.tile([P, 1], f32, tag="eps")
            nc.vector.memset(epsT[:, :], GN_EPS)
            # exercise every activation function used later so all ACT
            # tables load up-front (each mid-kernel ACT_TABLE_LOAD is ~1.3us
            # and stalls the ACT-dependent chain)
            actwarmT = cpool.tile([P, 1], f32, tag="actwarm")
            nc.scalar.copy(actwarmT[:, :], zeroT[:, :])
            for fn in (ident, silu, fsquare, fsqrt):
                nc.scalar.activation(actwarmT[:, :], zeroT[:, :], fn,
                                     bias=epsT[:, :], scale=1.0)
            if has_bv:
                bvbT = cpool.tile([P, NCH], f32, tag="bvb")
                nc.sync.dma_start(bvbT[:, :], bvb_d[:, :])

            # ---- persistent SBUF tensors ----
            qhT = big.tile([P, 2, S], bf16, tag="qh")
            khT = big.tile([P, 2, S], bf16, tag="kh")
            # khat zero-padded per head: head h=2mt+half lives in partition
            # rows 64*half of slot h, other 64 rows zero, so score matmuls
            # contract over the full 128 partitions (a single PE row position;
            # mixing 64-row tile positions within one PSUM bank locks up the
            # device).
            khzT = big.tile([P, 4, S], bf16, tag="khz")
            nc.vector.memset(khzT[:, :, :], 0.0)
            knT = big.tile([P, 2, NC, P], bf16, tag="kn")   # token-major khat
            vaT = big.tile([P, NC, NCH], bf16, tag="va")    # token-major v
            gateT = big.tile([P, 2, S], bf16, tag="gate")
            retT = big.tile([P, 2, S], bf16, tag="ret")
            gtdT = big.tile([P, 2, S], bf16, tag="gtd")
            statT = finp.tile([P, 16], f32, tag="stat")

            # ---- Phase A: Q/K projections + rotary (psw pipelined by 1) ----
            def _finish_qk(pend):
                dst, is_k, mt, c, t1, t2 = pend
                csl = slice(c * QCH, (c + 1) * QCH)
                psw = psmm.tile([P, QCH], f32, tag="mm")
                nc.tensor.matmul(psw[:, :], lhsT=ptT[:, :], rhs=t2[:, :],
                                 start=True, stop=True)
                nc.vector.tensor_tensor(dst[:, mt, csl], t1[:, :],
                                        psw[:, :], add_op)

            u32 = mybir.dt.uint32

            def _khz_copies(pend):
                _, is_k, mt, c = pend[:4]
                if not is_k:
                    return
                csl = slice(c * QCH, (c + 1) * QCH)
                # zero-padded per-head copies for 128-contract scores
                nc.scalar.copy(khzT[0:64, 2 * mt, csl],
                               khT[0:64, mt, csl])
                nc.scalar.copy(khzT[64:128, 2 * mt + 1, csl],
                               khT[64:128, mt, csl])

            # K then Q as one 16-unit stream sharing a depth-2 psw pipeline,
            # so neither projection's tail drains the PE
            pend = []
            done = []
            for nm, dst, cnm, snm, bnm, is_k in (
                    ("wk", khT, "ck", "sk", "bkr", True),
                    ("wq", qhT, "cq", "sq", "bqr", False)):
                for c in range(NQC):
                    for mt in range(2):
                        pst = psmm.tile([P, QCH], f32, tag="mm")
                        for kc in range(8):
                            nc.tensor.matmul(
                                pst[:, :],
                                lhsT=wT[nm][:, kc, mt * P:(mt + 1) * P],
                                rhs=xtT[:, c, kc, :],
                                start=(kc == 0), stop=(kc == 7))
                        if len(pend) >= 2:
                            done.append(pend.pop(0))
                            _finish_qk(done[-1])
                        qc = rotp.tile([P, QCH], bf16, tag="rt")
                        nc.scalar.activation(qc[:, :], pst[:, :], ident,
                                             bias=biasT[bnm][:, mt:mt + 1],
                                             scale=1.0)
                        csl = slice(c * QCH, (c + 1) * QCH)
                        t1 = rotp.tile([P, QCH], bf16, tag="rt")
                        nc.vector.tensor_tensor(t1[:, :], qc[:, :],
                                                csT[cnm][:, mt, csl], mul_op)
                        t2 = rotp.tile([P, QCH], bf16, tag="rt")
                        nc.gpsimd.tensor_tensor(t2[:, :], qc[:, :],
                                                csT[snm][:, mt, csl], mul_op)
                        pend.append((dst, is_k, mt, c, t1, t2))
                        if len(done) >= 2:
                            _khz_copies(done.pop(0))

            # ---- Phase A: V projection (token-major); the Q tail's psw
            # units flush under the first V matmul groups ----
            for tt in range(NC):
                pst = psmm.tile([P, QCH], f32, tag="mm")
                for kc in range(8):
                    nc.tensor.matmul(
                        pst[:, :NCH],
                        lhsT=xtT[:, tt // 4, kc,
                                 (tt % 4) * P:(tt % 4) * P + P],
                        rhs=wT["wv"][:, kc, :],
                        start=(kc == 0), stop=(kc == 7))
                if pend:
                    done.append(pend.pop(0))
                    _finish_qk(done[-1])
                elif done:
                    while done:
                        _khz_copies(done.pop(0))
                    # token-major khat: one blocked XBAR transpose per mt
                    # (needed first by retention, ~20us later)
                    for mt in range(2):
                        nc.sync.dma_start(knT[:, mt, :, :], khT[:, mt, :],
                                          transpose=True)
                if has_bv:
                    nc.vector.tensor_tensor(vaT[:, tt, :], pst[:, :NCH],
                                            bvbT[:, :], add_op)
                elif tt % 2 == 0:
                    nc.vector.tensor_copy(vaT[:, tt, :], pst[:, :NCH])
                else:
                    nc.scalar.copy(vaT[:, tt, :], pst[:, :NCH])

            # ---- Phase B: chunked retention, gate projection interleaved ----
            # State layout: [128 dk, 256] f32, block-diagonal per mt block:
            # head h=2mt+half occupies [64*half:+64, 128*mt+64*half:+64]; the
            # off-diagonal quadrants stay zero so the cross matmul can
            # contract over all 128 partitions in one instruction per mt.
            updT = psst.tile([P, 2 * P], f32, tag="state")
            nc.vector.memset(updT[:, :], 0.0)
            S32 = big.tile([P, 2 * P], f32, tag="s32")
            nc.vector.memset(S32[:, :], 0.0)

            def scores(c):
                ss = ssp.tile([P, QCH], bf16, tag="ss")
                sp = psmm.tile([P, QCH], f32, tag="mm")
                sl = slice(c * CK, (c + 1) * CK)
                for mt in range(2):
                    for half in range(2):
                        h = 2 * mt + half
                        nc.tensor.matmul(
                            sp[:, h * CK:(h + 1) * CK],
                            lhsT=khzT[:, h, sl],
                            rhs=qhT[:, mt, sl],
                            start=True, stop=True)
                nc.vector.tensor_tensor(ss[:, :], sp[:, :], triT[:, :],
                                        mul_op)
                return ss

            rp_cur = [None, None]

            def finish_chunk(cc, ss_cc, sb_cc):
                if cc % 4 == 0:
                    rp_cur[0] = psacc.tile([P, QCH], f32, tag="acc",
                                           name="rp0")
                    rp_cur[1] = psacc.tile([P, QCH], f32, tag="acc",
                                           name="rp1")
                rp = rp_cur
                base = (cc % 4) * CK
                qsl = slice(cc * CK, (cc + 1) * CK)
                # state update: S += khat_c^T v_c
                if cc < NC - 1:
                    for mt in range(2):
                        for half in range(2):
                            h = 2 * mt + half
                            pr = 64 * half
                            co = 128 * mt + 64 * half
                            nc.tensor.matmul(
                                updT[pr:pr + 64, co:co + 64],
                                lhsT=knT[:, mt, cc, pr:pr + 64],
                                rhs=vaT[:, cc, h * DH:(h + 1) * DH],
                                start=True, stop=True)
                    nc.vector.tensor_tensor(S32[:, :], S32[:, :],
                                            updT[:, :], add_op)
                # intra-chunk: rp += v^T ss
                for mt in range(2):
                    for half in range(2):
                        h = 2 * mt + half
                        pr = 64 * half
                        nc.tensor.matmul(
                            rp[mt][pr:pr + 64, base:base + CK],
                            lhsT=vaT[:, cc, h * DH:(h + 1) * DH],
                            rhs=ss_cc[:, h * CK:(h + 1) * CK],
                            start=True, stop=(cc == 0))
                # cross-chunk: rp += S^T qhat (S snapshot before this chunk)
                if cc > 0:
                    for mt in range(2):
                        nc.tensor.matmul(
                            rp[mt][:, base:base + CK],
                            lhsT=sb_cc[:, 128 * mt:128 * mt + 128],
                            rhs=qhT[:, mt, qsl],
                            start=False, stop=True)
                # snapshot state for next chunk's cross term
                sb_next = None
                if cc < NC - 1:
                    sb_next = sbp.tile([P, 2 * P], bf16, tag="sb")
                    nc.scalar.copy(sb_next[:, :], S32[:, :])
                # spill finished rp group + stats (sum via ACT accumulator,
                # sum-of-squares via DVE fused multiply-reduce on the spilled
                # bf16 copy -- avoids a 5th ACT function thrashing the tables)
                if cc % 4 == 3:
                    g = cc // 4
                    for mt in range(2):
                        sidx = mt * NQC + g
                        osl = slice(g * QCH, (g + 1) * QCH)
                        nc.scalar.activation(
                            retT[:, mt, osl], rp[mt][:, :], fcopy,
                            accum_out=statT[:, sidx:sidx + 1])
                        sqs = sqp.tile([P, QCH], bf16, tag="sq")
                        nc.scalar.activation(
                            sqs[:, :], rp[mt][:, :], fsquare,
                            bias=zeroT[:, :],
                            accum_out=statT[:, 8 + sidx:9 + sidx])
                return sb_next

            def gate_unit(g):
                mt, c = g // NQC, g % NQC
                pst = psmm.tile([P, QCH], f32, tag="mm")
                for kc in range(8):
                    nc.tensor.matmul(
                        pst[:, :],
                        lhsT=wT["wg"][:, kc, mt * P:(mt + 1) * P],
                        rhs=xtT[:, c, kc, :],
                        start=(kc == 0), stop=(kc == 7))
                nc.scalar.activation(
                    gateT[:, mt, c * QCH:(c + 1) * QCH], pst[:, :], silu,
                    bias=biasT["bgr"][:, mt:mt + 1], scale=1.0)

            import os as _os
            _n_ret = int(_os.environ.get("BISECT_NRET", str(NC)))
            if _n_ret < NC:
                nc.vector.memset(statT[:, :], 1.0)
                nc.vector.memset(retT[:, :, :], 0.0)
            # gates g0..g6 at odd chunks, g7 at c=14 so the final spills hit
            # an idle ACT queue; sqrt table pre-warms at c=13 so the
            # groupnorm chain doesn't pay the ACT_TABLE_LOAD
            gate_at = {1: 0, 3: 1, 5: 2, 7: 3, 9: 4, 11: 5, 13: 6, 14: 7}
            ss_prev = None
            sb_prev = None
            done_gates = set()
            for c in range(_n_ret + 1):
                ss_cur = scores(c) if c < _n_ret else None
                if c >= 1:
                    sb_prev = finish_chunk(c - 1, ss_prev, sb_prev)
                ss_prev = ss_cur
                if c == 13:
                    nc.scalar.activation(actwarmT[:, :], zeroT[:, :], fsqrt,
                                         bias=epsT[:, :], scale=1.0)
                g = gate_at.get(c)
                if g is not None:
                    gate_unit(g)
                    done_gates.add(g)
            for g in range(8):
                if g not in done_gates:
                    gate_unit(g)

            # ---- Phase C: groupnorm finalize + gate + output projection ----
            s1 = finp.tile([P, 4], f32, tag="s1")
            nc.vector.tensor_reduce(
                s1[:, :], statT[:, :].rearrange("p (g c) -> p g c", c=NQC),
                axis=mybir.AxisListType.X, op=add_op)
            s1b = finp.tile([P, 4], bf16, tag="s1b")
            nc.vector.tensor_copy(s1b[:, :], s1[:, :])
            totp = psmm.tile([P, QCH], f32, tag="mm")
            nc.tensor.matmul(totp[:, :4], lhsT=obT[:, :], rhs=s1b[:, :],
                             start=True, stop=True)
            # tot = [sum, sumsq] per head; scale to [mean, E[x^2]] in one op
            tot = finp.tile([P, 4], f32, tag="tot_sb")
            nc.vector.tensor_scalar_mul(tot[:, :], totp[:, :4], 1.0 / NELEM)
            mean = tot[:, 0:2]
            msq = finp.tile([P, 2], f32, tag="msq")
            nc.vector.tensor_tensor(msq[:, :], mean, mean, mul_op)
            var = finp.tile([P, 2], f32, tag="var")
            nc.vector.tensor_tensor(var[:, :], tot[:, 2:4], msq[:, :], sub_op)
            std = finp.tile([P, 2], f32, tag="std")
            nc.scalar.activation(std[:, :], var[:, :], fsqrt,
                                 bias=epsT[:, :], scale=1.0)
            istd = finp.tile([P, 2], f32, tag="istd")
            nc.vector.reciprocal(istd[:, :], std[:, :])
            aff_a = finp.tile([P, 2], f32, tag="aff_a")
            nc.vector.tensor_tensor(aff_a[:, :], istd[:, :], gnwT[:, :], mul_op)
            ma = finp.tile([P, 2], f32, tag="ma")
            nc.vector.tensor_tensor(ma[:, :], mean, aff_a[:, :], mul_op)
            aff_b = finp.tile([P, 2], f32, tag="aff_b")
            nc.vector.tensor_tensor(aff_b[:, :], gnbT[:, :], ma[:, :], sub_op)

            out_re = out_d.ap().rearrange("(a p) d -> p a d", p=P)
            cp_i = 0
            for c in range(NQC):
                # split c=0 in halves so the first output-projection group
                # starts ~0.5us earlier off the groupnorm critical chain
                parts = ((0, QCH // 2), (QCH // 2, QCH)) if c == 0 \
                    else ((0, QCH),)
                for mt in range(2):
                    nrm = nrmp.tile([P, QCH], bf16, tag="nrm")
                    for lo, hi in parts:
                        osl = slice(c * QCH + lo, c * QCH + hi)
                        nc.scalar.activation(nrm[:, lo:hi], retT[:, mt, osl],
                                             ident,
                                             bias=aff_b[:, mt:mt + 1],
                                             scale=aff_a[:, mt:mt + 1])
                        nc.vector.tensor_tensor(gtdT[:, mt, osl],
                                                nrm[:, lo:hi],
                                                gateT[:, mt, osl], mul_op)
                for pair in range(2):
                    tt0 = 4 * c + 2 * pair
                    ob_t = outp.tile([P, 2, D], bf16, tag="ob")
                    for j in range(2):
                        tt = tt0 + j
                        for oc in range(2):
                            op_ps = psacc.tile([P, QCH], f32, tag="acc")
                            for kc in range(2):
                                nc.tensor.matmul(
                                    op_ps[:, :],
                                    lhsT=gtdT[:, kc, tt * P:(tt + 1) * P],
                                    rhs=woT[:, kc, oc * QCH:(oc + 1) * QCH],
                                    start=(kc == 0), stop=(kc == 1))
                            dst = ob_t[:, j, oc * QCH:(oc + 1) * QCH]
                            if cp_i % 2 == 0:
                                nc.vector.tensor_copy(dst, op_ps[:, :])
                            else:
                                nc.scalar.copy(dst, op_ps[:, :])
                            cp_i += 1
                    nc.sync.dma_start(out_re[:, tt0:tt0 + 2, :],
                                      ob_t[:, :, :])

    nc.compile()
    return nc


_PROGRAM_CACHE = {}


def _get_program(has_bv):
    if has_bv not in _PROGRAM_CACHE:
        _PROGRAM_CACHE[has_bv] = _build_program(has_bv)
    return _PROGRAM_CACHE[has_bv]


def kernel(**inputs):
    global LAST_EXEC_NS, LAST_RESULTS
    in_maps, has_bv = _host_prep(inputs)
    nc = _get_program(has_bv)
    trace = bool(int(os.environ.get("KERNEL_TRACE", "0")))
    kw = {}
    if trace:
        kw["trace"] = True
        kw["trace_cores"] = [int(c) for c in
                             os.environ.get("KERNEL_TRACE_CORES", "0").split(",")]
        td = os.environ.get("KERNEL_TRACE_DIR")
        if td:
            os.makedirs(td, exist_ok=True)
            kw["tmpdir"] = td
    res = run_bass_kernel_spmd(nc, in_maps, list(range(N_CORES)), **kw)
    LAST_EXEC_NS = res.exec_time_ns
    LAST_RESULTS = res
    bo = np.asarray(inputs["bo"], np.float32)
    out = np.zeros((B, S, D), np.float32)
    for core in range(N_CORES):
        out[core // HG] += res.results[core]["out"].astype(np.float32)
    out += bo[None, None, :]
    return out


# revision 70
# speedup vs baseline: 2.5491x; 1.0370x over previous
"""Trainium2 Bass kernel for GatedMultiScaleRetentionLayer.

Sharding: 8 cores = data-parallel over batch (B=2) x tensor-parallel over
heads (16 heads -> 4 groups of 4). Each core computes its batch's tokens for
its 4 heads end-to-end; host sums the 4 bf16 partial outputs per batch and
adds bo.

Retention uses the chunked-state form instead of materializing all causal
score blocks: with decay folded per token (qhat_i = rot(q_i) gamma^i rownorm_i,
khat_j = rot(k_j) gamma^-j), the running state S = sum_j khat_j v_j^T
(accumulated in PSUM across 128-token chunks) turns all cross-chunk history
into one [64,64] matmul per head per chunk. Only the intra-chunk 128x128
block needs explicit scores + triangular masking. This cuts retention PE
work ~4x vs the all-blocks form.

The interleaved rotary is made partition-friendly by permuting the rotary
dims of Wq/Wk columns (even indices first), so rotate_half becomes a fixed
16-row block swap done with one small PE matmul against a +-1 permutation
matrix. khat in token-major layout (for the state update) is produced by
SBUF->SBUF DMA XBAR transposes.
"""

import os

import numpy as np
import ml_dtypes

import concourse.bass as bass
import concourse.tile as tile
from concourse import bacc, mybir
from concourse.bass_utils import run_bass_kernel_spmd

BF16 = ml_dtypes.bfloat16

# ---- problem constants (hardcoded per contract) ----
B = 2
S = 2048
D = 1024
H = 16
DH = 64
ROT = 32
THETA = 10000.0
XPOS_BASE = 512.0
GN_EPS = 1e-5

N_CORES = 8
HG = 4          # head groups (tensor-parallel)
HPC = 4         # heads per core
NCH = HPC * DH  # 256 channels per core
P = 128
QCH = 512       # 512-token projection chunk (one PSUM bank of fp32)
NQC = S // QCH  # 4 projection chunks per mt
CK = 128        # retention chunk
NC = S // CK    # 16 retention chunks
NELEM = float(S * DH)  # groupnorm element count per head

LAST_EXEC_NS = None
LAST_RESULTS = None

_PERM = np.concatenate([np.arange(0, ROT, 2), np.arange(1, ROT, 2),
                        np.arange(ROT, DH)])  # within-head column permutation


def _perm_cols(w_slice):
    """Permute rotary dims of each head's 64-column block (even idx first)."""
    out = np.empty_like(w_slice)
    for hl in range(HPC):
        blk = w_slice[..., hl * DH:(hl + 1) * DH]
        out[..., hl * DH:(hl + 1) * DH] = blk[..., _PERM]
    return out


def _rot_tables():
    """angle[t, r], xpos scale[t, r] for pair index r in [0,16)."""
    t = np.arange(S, dtype=np.float64)
    r = np.arange(ROT // 2, dtype=np.float64)
    inv_freq = 1.0 / (THETA ** ((2.0 * r) / ROT))
    angle = t[:, None] * inv_freq[None, :]                   # [S, 16]
    base = (2.0 * r + 0.4 * ROT) / (1.4 * ROT)               # [16]
    power = (t - S // 2) / XPOS_BASE                         # [S]
    scale = base[None, :] ** power[:, None]                  # [S, 16]
    return angle, scale


def _decay_factors():
    """gamma^i*rownorm (for q) and gamma^-j (for k), per global head. f64."""
    h = np.arange(H, dtype=np.float64)
    gamma = 1.0 - 2.0 ** (-5.0 - h)                          # [H]
    t = np.arange(S, dtype=np.float64)
    logg = np.log(gamma)
    g_pos = np.exp(t[None, :] * logg[:, None])               # [H, S] gamma^t
    g_neg = np.exp(-t[None, :] * logg[:, None])              # [H, S] gamma^-t
    rowsum = (1.0 - gamma[:, None] * g_pos) / (1.0 - gamma[:, None])
    rownorm = 1.0 / np.sqrt(rowsum)                          # [H, S]
    return g_pos * rownorm, g_neg


def _cs_tiles(h0):
    """cq, sq, ck, sk tiles [2, 128, S] bf16 for heads h0..h0+3."""
    angle, scale = _rot_tables()
    dq_all, dk_all = _decay_factors()
    cos, sin = np.cos(angle), np.sin(angle)                  # [S, 16]
    cq = np.zeros((2, P, S), np.float64)
    sq = np.zeros((2, P, S), np.float64)
    ck = np.zeros((2, P, S), np.float64)
    sk = np.zeros((2, P, S), np.float64)
    for mt in range(2):
        for half in range(2):
            hl = 2 * mt + half
            g = h0 + hl
            dq = dq_all[g]                                   # [S]
            dk = dk_all[g]
            base = 64 * half
            for rr in range(16):
                cq[mt, base + rr] = cos[:, rr] * scale[:, rr] * dq
                cq[mt, base + 16 + rr] = cos[:, rr] * scale[:, rr] * dq
                sq[mt, base + rr] = sin[:, rr] * scale[:, rr] * dq
                sq[mt, base + 16 + rr] = sin[:, rr] * scale[:, rr] * dq
                ck[mt, base + rr] = cos[:, rr] / scale[:, rr] * dk
                ck[mt, base + 16 + rr] = cos[:, rr] / scale[:, rr] * dk
                sk[mt, base + rr] = sin[:, rr] / scale[:, rr] * dk
                sk[mt, base + 16 + rr] = sin[:, rr] / scale[:, rr] * dk
            cq[mt, base + 32:base + 64] = dq[None, :]
            ck[mt, base + 32:base + 64] = dk[None, :]
    return (cq.astype(BF16), sq.astype(BF16), ck.astype(BF16), sk.astype(BF16))


def _pt_matrix():
    """lhsT of the rotate-half block-swap matrix (out = P @ rhs)."""
    Pm = np.zeros((P, P), np.float32)
    for b0 in (0, 64):
        for rr in range(16):
            Pm[b0 + rr, b0 + 16 + rr] = -1.0
            Pm[b0 + 16 + rr, b0 + rr] = 1.0
    return Pm.T.astype(BF16)  # Pt[k, m] = P[m, k]


def _tri_mask():
    tri = (np.arange(P)[None, :] >= np.arange(P)[:, None])
    return np.tile(tri, (1, 4)).astype(BF16)  # tri[rj, t%128] = t >= rj, x4


def _blockones():
    k = np.arange(P)
    return (k[:, None] // 64 == k[None, :] // 64).astype(BF16)


def _rep2(vec_slice):
    """[256] channel vector -> [128, 2] f32 (per-partition, per m-tile)."""
    out = np.empty((P, 2), np.float32)
    for mt in range(2):
        out[:, mt] = vec_slice[mt * P:(mt + 1) * P]
    return out


def _host_prep(inputs):
    x = np.asarray(inputs["x"], np.float32)
    Wq = np.asarray(inputs["Wq"], np.float32)
    Wk = np.asarray(inputs["Wk"], np.float32)
    Wv = np.asarray(inputs["Wv"], np.float32)
    Wg = np.asarray(inputs["Wg"], np.float32)
    Wo = np.asarray(inputs["Wo"], np.float32)
    bq = np.asarray(inputs["bq"], np.float32)
    bk = np.asarray(inputs["bk"], np.float32)
    bv = np.asarray(inputs["bv"], np.float32)
    bg = np.asarray(inputs["bg"], np.float32)
    gn_w = np.asarray(inputs["gn_w"], np.float32)
    gn_b = np.asarray(inputs["gn_b"], np.float32)

    pt = _pt_matrix()
    tri = _tri_mask()
    ob = _blockones()
    has_bv = bool(np.any(bv != 0.0))

    in_maps = []
    for core in range(N_CORES):
        b = core // HG
        hg = core % HG
        h0 = HPC * hg
        cols = slice(NCH * hg, NCH * (hg + 1))
        cq, sq, ck, sk = _cs_tiles(h0)
        gnw_rep = np.empty((P, 2), np.float32)
        gnb_rep = np.empty((P, 2), np.float32)
        for mt in range(2):
            for half in range(2):
                g = h0 + 2 * mt + half
                gnw_rep[64 * half:64 * (half + 1), mt] = gn_w[g]
                gnb_rep[64 * half:64 * (half + 1), mt] = gn_b[g]
        def pmaj_w(w, nb=8):
            # [nb*P, N] -> [P, nb, N] so each partition's data is contiguous
            return np.ascontiguousarray(
                w.reshape(nb, P, -1).transpose(1, 0, 2)).astype(BF16)

        xt = x[b].T.reshape(8, P, S).transpose(1, 0, 2)      # [P, 8, S]
        m = {
            "wq": pmaj_w(_perm_cols(Wq[:, cols])),
            "wk": pmaj_w(_perm_cols(Wk[:, cols])),
            "wv": pmaj_w(Wv[:, cols]),
            "wg": pmaj_w(Wg[:, cols]),
            "wo": pmaj_w(Wo[cols, :], nb=2),
            "cq": np.ascontiguousarray(cq.transpose(1, 0, 2)),
            "sq": np.ascontiguousarray(sq.transpose(1, 0, 2)),
            "ck": np.ascontiguousarray(ck.transpose(1, 0, 2)),
            "sk": np.ascontiguousarray(sk.transpose(1, 0, 2)),
            "pt": pt, "tri": tri, "ob": ob,
            "gnw": gnw_rep, "gnb": gnb_rep,
            "bqr": _rep2(_perm_cols(bq[None, cols])[0]),
            "bkr": _rep2(_perm_cols(bk[None, cols])[0]),
            "bgr": _rep2(bg[cols]),
        }
        for cc4 in range(4):
            m[f"xt{cc4}"] = np.ascontiguousarray(
                xt[:, :, cc4 * QCH:(cc4 + 1) * QCH]).astype(BF16)
        if has_bv:
            m["bvb"] = np.broadcast_to(bv[cols][None, :], (P, NCH)).astype(
                np.float32).copy()
        in_maps.append(m)
    return in_maps, has_bv


def _build_program(has_bv):
    nc = bacc.Bacc("TRN2", target_bir_lowering=False, debug=False,
                   num_devices=N_CORES)
    f32 = mybir.dt.float32
    bf16 = mybir.dt.bfloat16

    xt_ds = [nc.dram_tensor(f"xt{c}", [P, 8, QCH], bf16, kind="ExternalInput")
             for c in range(4)]
    wq_d = nc.dram_tensor("wq", [P, 8, NCH], bf16, kind="ExternalInput")
    wk_d = nc.dram_tensor("wk", [P, 8, NCH], bf16, kind="ExternalInput")
    wv_d = nc.dram_tensor("wv", [P, 8, NCH], bf16, kind="ExternalInput")
    wg_d = nc.dram_tensor("wg", [P, 8, NCH], bf16, kind="ExternalInput")
    wo_d = nc.dram_tensor("wo", [P, 2, D], bf16, kind="ExternalInput")
    cq_d = nc.dram_tensor("cq", [P, 2, S], bf16, kind="ExternalInput")
    sq_d = nc.dram_tensor("sq", [P, 2, S], bf16, kind="ExternalInput")
    ck_d = nc.dram_tensor("ck", [P, 2, S], bf16, kind="ExternalInput")
    sk_d = nc.dram_tensor("sk", [P, 2, S], bf16, kind="ExternalInput")
    pt_d = nc.dram_tensor("pt", [P, P], bf16, kind="ExternalInput")
    tri_d = nc.dram_tensor("tri", [P, 4 * P], bf16, kind="ExternalInput")
    ob_d = nc.dram_tensor("ob", [P, P], bf16, kind="ExternalInput")
    gnw_d = nc.dram_tensor("gnw", [P, 2], f32, kind="ExternalInput")
    gnb_d = nc.dram_tensor("gnb", [P, 2], f32, kind="ExternalInput")
    bqr_d = nc.dram_tensor("bqr", [P, 2], f32, kind="ExternalInput")
    bkr_d = nc.dram_tensor("bkr", [P, 2], f32, kind="ExternalInput")
    bgr_d = nc.dram_tensor("bgr", [P, 2], f32, kind="ExternalInput")
    bvb_d = (nc.dram_tensor("bvb", [P, NCH], f32, kind="ExternalInput")
             if has_bv else None)
    out_d = nc.dram_tensor("out", [S, D], bf16, kind="ExternalOutput")

    ident = mybir.ActivationFunctionType.Identity
    silu = mybir.ActivationFunctionType.Silu
    fcopy = mybir.ActivationFunctionType.Copy
    fsquare = mybir.ActivationFunctionType.Square
    fsqrt = mybir.ActivationFunctionType.Sqrt
    mul_op = mybir.AluOpType.mult
    add_op = mybir.AluOpType.add
    sub_op = mybir.AluOpType.subtract

    with tile.TileContext(nc) as tc:
        with (
            tc.tile_pool(name="consts", bufs=1) as cpool,
            tc.tile_pool(name="wts", bufs=1) as wpool,
            tc.tile_pool(name="big", bufs=1) as big,
            tc.tile_pool(name="mm", bufs=3, space="PSUM") as psmm,
            tc.tile_pool(name="acc", bufs=4, space="PSUM") as psacc,
            tc.tile_pool(name="state", bufs=1, space="PSUM") as psst,
            tc.tile_pool(name="rotp", bufs=9) as rotp,
            tc.tile_pool(name="ssp", bufs=10) as ssp,
            tc.tile_pool(name="sbp", bufs=2) as sbp,
            tc.tile_pool(name="sqp", bufs=2) as sqp,
            tc.tile_pool(name="nrmp", bufs=4) as nrmp,
            tc.tile_pool(name="outp", bufs=4) as outp,
            tc.tile_pool(name="finp", bufs=1) as finp,
        ):
            # ---- input DMAs: all host-prelayouted to partition-major so
            # every descriptor is one contiguous row per partition (cheap to
            # generate), ordered critical-first ----
            wT = {}
            for nm in ("wq", "wk", "wv", "wg"):
                wT[nm] = wpool.tile([P, 8, NCH], bf16, tag=nm, name=nm)
            xtT = big.tile([P, 4, 8, QCH], bf16, tag="xt")
            csT = {}
            for nm in ("cq", "sq", "ck", "sk"):
                csT[nm] = cpool.tile([P, 2, S], bf16, tag=nm, name=nm)

            nc.sync.dma_start(wT["wk"][:, :, :], wk_d[:, :, :])
            nc.sync.dma_start(xtT[:, 0, :, :], xt_ds[0][:, :, :])
            biasT = {}
            for nm, dh in (("bqr", bqr_d), ("bkr", bkr_d)):
                t = cpool.tile([P, 2], f32, tag=nm, name=nm)
                nc.sync.dma_start(t[:, :], dh[:, :])
                biasT[nm] = t
            def cs_chunk(nm, dh, c):
                sl = slice(c * QCH, (c + 1) * QCH)
                nc.sync.dma_start(csT[nm][:, :, sl], dh[:, :, sl])

            cs_chunk("ck", ck_d, 0)
            cs_chunk("sk", sk_d, 0)
            ptT = cpool.tile([P, P], bf16, tag="pt")
            nc.sync.dma_start(ptT[:, :], pt_d[:, :])
            nc.sync.dma_start(xtT[:, 1, :, :], xt_ds[1][:, :, :])
            cs_chunk("ck", ck_d, 1)
            cs_chunk("sk", sk_d, 1)
            nc.sync.dma_start(xtT[:, 2, :, :], xt_ds[2][:, :, :])
            cs_chunk("ck", ck_d, 2)
            cs_chunk("sk", sk_d, 2)
            nc.sync.dma_start(wT["wq"][:, :, :], wq_d[:, :, :])
            nc.sync.dma_start(xtT[:, 3, :, :], xt_ds[3][:, :, :])
            cs_chunk("ck", ck_d, 3)
            cs_chunk("sk", sk_d, 3)
            for c in range(NQC):
                cs_chunk("cq", cq_d, c)
                cs_chunk("sq", sq_d, c)
            triT = cpool.tile([P, 4 * P], bf16, tag="tri")
            nc.sync.dma_start(triT[:, :], tri_d[:, :])
            nc.sync.dma_start(wT["wv"][:, :, :], wv_d[:, :, :])
            nc.sync.dma_start(wT["wg"][:, :, :], wg_d[:, :, :])
            woT = wpool.tile([P, 2, D], bf16, tag="wo")
            nc.sync.dma_start(woT[:, :, :], wo_d[:, :, :])
            obT = cpool.tile([P, P], bf16, tag="ob")
            nc.sync.dma_start(obT[:, :], ob_d[:, :])
            gnwT = cpool.tile([P, 2], f32, tag="gnw")
            nc.sync.dma_start(gnwT[:, :], gnw_d[:, :])
            gnbT = cpool.tile([P, 2], f32, tag="gnb")
            nc.sync.dma_start(gnbT[:, :], gnb_d[:, :])
            t = cpool.tile([P, 2], f32, tag="bgr", name="bgr")
            nc.sync.dma_start(t[:, :], bgr_d[:, :])
            biasT["bgr"] = t
            zeroT = cpool.tile([P, 1], f32, tag="zero")
            nc.vector.memset(zeroT[:, :], 0.0)
            epsT = cpool.tile([P, 1], f32, tag="eps")
            nc.vector.memset(epsT[:, :], GN_EPS)
            # exercise every activation function used later so all ACT
            # tables load up-front (each mid-kernel ACT_TABLE_LOAD is ~1.3us
            # and stalls the ACT-dependent chain)
            actwarmT = cpool.tile([P, 1], f32, tag="actwarm")
            nc.scalar.copy(actwarmT[:, :], zeroT[:, :])
            for fn in (ident, silu, fsquare, fsqrt):
                nc.scalar.activation(actwarmT[:, :], zeroT[:, :], fn,
                                     bias=epsT[:, :], scale=1.0)
            if has_bv:
                bvbT = cpool.tile([P, NCH], f32, tag="bvb")
                nc.sync.dma_start(bvbT[:, :], bvb_d[:, :])

            # ---- persistent SBUF tensors ----
            qhT = big.tile([P, 2, S], bf16, tag="qh")
            khT = big.tile([P, 2, S], bf16, tag="kh")
            # khat zero-padded per head: head h=2mt+half lives in partition
            # rows 64*half of slot h, other 64 rows zero, so score matmuls
            # contract over the full 128 partitions (a single PE row position;
            # mixing 64-row tile positions within one PSUM bank locks up the
            # device).
            khzT = big.tile([P, 4, S], bf16, tag="khz")
            nc.vector.memset(khzT[:, :, :], 0.0)
            knT = big.tile([P, 2, NC, P], bf16, tag="kn")   # token-major khat
            vaT = big.tile([P, NC, NCH], bf16, tag="va")    # token-major v
            gateT = big.tile([P, 2, S], bf16, tag="gate")
            retT = big.tile([P, 2, S], bf16, tag="ret")
            gtdT = big.tile([P, 2, S], bf16, tag="gtd")
            statT = finp.tile([P, 16], f32, tag="stat")

            # ---- Phase A: Q/K projections + rotary (psw pipelined by 1) ----
            def _finish_qk(pend):
                dst, is_k, mt, c, t1, t2 = pend
                csl = slice(c * QCH, (c + 1) * QCH)
                psw = psmm.tile([P, QCH], f32, tag="mm")
                nc.tensor.matmul(psw[:, :], lhsT=ptT[:, :], rhs=t2[:, :],
                                 start=True, stop=True)
                nc.vector.tensor_tensor(dst[:, mt, csl], t1[:, :],
                                        psw[:, :], add_op)

            u32 = mybir.dt.uint32

            def _khz_copies(pend):
                _, is_k, mt, c = pend[:4]
                if not is_k:
                    return
                csl = slice(c * QCH, (c + 1) * QCH)
                # zero-padded per-head copies for 128-contract scores
                nc.scalar.copy(khzT[0:64, 2 * mt, csl],
                               khT[0:64, mt, csl])
                nc.scalar.copy(khzT[64:128, 2 * mt + 1, csl],
                               khT[64:128, mt, csl])

            # K then Q as one 16-unit stream sharing a depth-2 psw pipeline,
            # so neither projection's tail drains the PE
            pend = []
            done = []
            for nm, dst, cnm, snm, bnm, is_k in (
                    ("wk", khT, "ck", "sk", "bkr", True),
                    ("wq", qhT, "cq", "sq", "bqr", False)):
                for c in range(NQC):
                    for mt in range(2):
                        pst = psmm.tile([P, QCH], f32, tag="mm")
                        for kc in range(8):
                            nc.tensor.matmul(
                                pst[:, :],
                                lhsT=wT[nm][:, kc, mt * P:(mt + 1) * P],
                                rhs=xtT[:, c, kc, :],
                                start=(kc == 0), stop=(kc == 7))
                        if len(pend) >= 2:
                            done.append(pend.pop(0))
                            _finish_qk(done[-1])
                        qc = rotp.tile([P, QCH], bf16, tag="rt")
                        nc.scalar.activation(qc[:, :], pst[:, :], ident,
                                             bias=biasT[bnm][:, mt:mt + 1],
                                             scale=1.0)
                        csl = slice(c * QCH, (c + 1) * QCH)
                        t1 = rotp.tile([P, QCH], bf16, tag="rt")
                        nc.vector.tensor_tensor(t1[:, :], qc[:, :],
                                                csT[cnm][:, mt, csl], mul_op)
                        t2 = rotp.tile([P, QCH], bf16, tag="rt")
                        nc.gpsimd.tensor_tensor(t2[:, :], qc[:, :],
                                                csT[snm][:, mt, csl], mul_op)
                        pend.append((dst, is_k, mt, c, t1, t2))
                        if len(done) >= 2:
                            _khz_copies(done.pop(0))

            # ---- Phase A: V projection (token-major); the Q tail's psw
            # units flush under the first V matmul groups ----
            for tt in range(NC):
                pst = psmm.tile([P, QCH], f32, tag="mm")
                for kc in range(8):
                    nc.tensor.matmul(
                        pst[:, :NCH],
                        lhsT=xtT[:, tt // 4, kc,
                                 (tt % 4) * P:(tt % 4) * P + P],
                        rhs=wT["wv"][:, kc, :],
                        start=(kc == 0), stop=(kc == 7))
                if pend:
                    done.append(pend.pop(0))
                    _finish_qk(done[-1])
                elif done:
                    while done:
                        _khz_copies(done.pop(0))
                    # token-major khat: one blocked XBAR transpose per mt
                    # (needed first by retention, ~20us later)
                    for mt in range(2):
                        nc.sync.dma_start(knT[:, mt, :, :], khT[:, mt, :],
                                          transpose=True)
                if has_bv:
                    nc.vector.tensor_tensor(vaT[:, tt, :], pst[:, :NCH],
                                            bvbT[:, :], add_op)
                elif tt % 2 == 0:
                    nc.vector.tensor_copy(vaT[:, tt, :], pst[:, :NCH])
                else:
                    nc.scalar.copy(vaT[:, tt, :], pst[:, :NCH])

            # ---- Phase B: chunked retention, gate projection interleaved ----
            # State layout: [128 dk, 256] f32, block-diagonal per mt block:
            # head h=2mt+half occupies [64*half:+64, 128*mt+64*half:+64]; the
            # off-diagonal quadrants stay zero so the cross matmul can
            # contract over all 128 partitions in one instruction per mt.
            updT = psst.tile([P, 2 * P], f32, tag="state")
            nc.vector.memset(updT[:, :], 0.0)
            S32 = big.tile([P, 2 * P], f32, tag="s32")
            nc.vector.memset(S32[:, :], 0.0)

            def scores(c):
                ss = ssp.tile([P, QCH], bf16, tag="ss")
                sp = psmm.tile([P, QCH], f32, tag="mm")
                sl = slice(c * CK, (c + 1) * CK)
                for mt in range(2):
                    for half in range(2):
                        h = 2 * mt + half
                        nc.tensor.matmul(
                            sp[:, h * CK:(h + 1) * CK],
                            lhsT=khzT[:, h, sl],
                            rhs=qhT[:, mt, sl],
                            start=True, stop=True)
                nc.vector.tensor_tensor(ss[:, :], sp[:, :], triT[:, :],
                                        mul_op)
                return ss

            rp_cur = [None, None]

            def finish_chunk(cc, ss_cc, sb_cc):
                if cc % 4 == 0:
                    rp_cur[0] = psacc.tile([P, QCH], f32, tag="acc",
                                           name="rp0")
                    rp_cur[1] = psacc.tile([P, QCH], f32, tag="acc",
                                           name="rp1")
                rp = rp_cur
                base = (cc % 4) * CK
                qsl = slice(cc * CK, (cc + 1) * CK)
                # state update: S += khat_c^T v_c
                if cc < NC - 1:
                    for mt in range(2):
                        for half in range(2):
                            h = 2 * mt + half
                            pr = 64 * half
                            co = 128 * mt + 64 * half
                            nc.tensor.matmul(
                                updT[pr:pr + 64, co:co + 64],
                                lhsT=knT[:, mt, cc, pr:pr + 64],
                                rhs=vaT[:, cc, h * DH:(h + 1) * DH],
                                start=True, stop=True)
                    nc.vector.tensor_tensor(S32[:, :], S32[:, :],
                                            updT[:, :], add_op)
                # intra-chunk: rp += v^T ss
                for mt in range(2):
                    for half in range(2):
                        h = 2 * mt + half
                        pr = 64 * half
                        nc.tensor.matmul(
                            rp[mt][pr:pr + 64, base:base + CK],
                            lhsT=vaT[:, cc, h * DH:(h + 1) * DH],
                            rhs=ss_cc[:, h * CK:(h + 1) * CK],
                            start=True, stop=(cc == 0))
                # cross-chunk: rp += S^T qhat (S snapshot before this chunk)
                if cc > 0:
                    for mt in range(2):
                        nc.tensor.matmul(
                            rp[mt][:, base:base + CK],
                            lhsT=sb_cc[:, 128 * mt:128 * mt + 128],
                            rhs=qhT[:, mt, qsl],
                            start=False, stop=True)
                # snapshot state for next chunk's cross term
                sb_next = None
                if cc < NC - 1:
                    sb_next = sbp.tile([P, 2 * P], bf16, tag="sb")
                    nc.scalar.copy(sb_next[:, :], S32[:, :])
                # spill finished rp group + stats (sum via ACT accumulator,
                # sum-of-squares via DVE fused multiply-reduce on the spilled
                # bf16 copy -- avoids a 5th ACT function thrashing the tables)
                if cc % 4 == 3:
                    g = cc // 4
                    for mt in range(2):
                        sidx = mt * NQC + g
                        osl = slice(g * QCH, (g + 1) * QCH)
                        nc.scalar.activation(
                            retT[:, mt, osl], rp[mt][:, :], fcopy,
                            accum_out=statT[:, sidx:sidx + 1])
                        sqs = sqp.tile([P, QCH], bf16, tag="sq")
                        nc.scalar.activation(
                            sqs[:, :], rp[mt][:, :], fsquare,
                            bias=zeroT[:, :],
                            accum_out=statT[:, 8 + sidx:9 + sidx])
                return sb_next

            def gate_unit(g):
                mt, c = g // NQC, g % NQC
                pst = psmm.tile([P, QCH], f32, tag="mm")
                for kc in range(8):
                    nc.tensor.matmul(
                        pst[:, :],
                        lhsT=wT["wg"][:, kc, mt * P:(mt + 1) * P],
                        rhs=xtT[:, c, kc, :],
                        start=(kc == 0), stop=(kc == 7))
                nc.scalar.activation(
                    gateT[:, mt, c * QCH:(c + 1) * QCH], pst[:, :], silu,
                    bias=biasT["bgr"][:, mt:mt + 1], scale=1.0)

            import os as _os
            _n_ret = int(_os.environ.get("BISECT_NRET", str(NC)))
            if _n_ret < NC:
                nc.vector.memset(statT[:, :], 1.0)
                nc.vector.memset(retT[:, :, :], 0.0)
            # gates g0..g6 at odd chunks, g7 at c=14 so the final spills hit
            # an idle ACT queue; sqrt table pre-warms at c=13 so the
            # groupnorm chain doesn't pay the ACT_TABLE_LOAD
            gate_at = {1: 0, 3: 1, 5: 2, 7: 3, 9: 4, 11: 5, 13: 6, 14: 7}
            ss_prev = None
            sb_prev = None
            done_gates = set()
            for c in range(_n_ret + 1):
                ss_cur = scores(c) if c < _n_ret else None
                if c >= 1:
                    sb_prev = finish_chunk(c - 1, ss_prev, sb_prev)
                ss_prev = ss_cur
                if c == 13:
                    nc.scalar.activation(actwarmT[:, :], zeroT[:, :], fsqrt,
                                         bias=epsT[:, :], scale=1.0)
                g = gate_at.get(c)
                if g is not None:
                    gate_unit(g)
                    done_gates.add(g)
            for g in range(8):
                if g not in done_gates:
                    gate_unit(g)

            # ---- Phase C: groupnorm finalize + gate + output projection ----
            s1 = finp.tile([P, 4], f32, tag="s1")
            nc.vector.tensor_reduce(
                s1[:, :], statT[:, :].rearrange("p (g c) -> p g c", c=NQC),
                axis=mybir.AxisListType.X, op=add_op)
            s1b = finp.tile([P, 4], bf16, tag="s1b")
            nc.vector.tensor_copy(s1b[:, :], s1[:, :])
            totp = psmm.tile([P, QCH], f32, tag="mm")
            nc.tensor.matmul(totp[:, :4], lhsT=obT[:, :], rhs=s1b[:, :],
                             start=True, stop=True)
            # tot = [sum, sumsq] per head; scale to [mean, E[x^2]] in one op
            tot = finp.tile([P, 4], f32, tag="tot_sb")
            nc.vector.tensor_scalar_mul(tot[:, :], totp[:, :4], 1.0 / NELEM)
            mean = tot[:, 0:2]
            msq = finp.tile([P, 2], f32, tag="msq")
            nc.vector.tensor_tensor(msq[:, :], mean, mean, mul_op)
            var = finp.tile([P, 2], f32, tag="var")
            nc.vector.tensor_tensor(var[:, :], tot[:, 2:4], msq[:, :], sub_op)
            std = finp.tile([P, 2], f32, tag="std")
            nc.scalar.activation(std[:, :], var[:, :], fsqrt,
                                 bias=epsT[:, :], scale=1.0)
            istd = finp.tile([P, 2], f32, tag="istd")
            nc.vector.reciprocal(istd[:, :], std[:, :])
            aff_a = finp.tile([P, 2], f32, tag="aff_a")
            nc.vector.tensor_tensor(aff_a[:, :], istd[:, :], gnwT[:, :], mul_op)
            ma = finp.tile([P, 2], f32, tag="ma")
            nc.vector.tensor_tensor(ma[:, :], mean, aff_a[:, :], mul_op)
            aff_b = finp.tile([P, 2], f32, tag="aff_b")
            nc.vector.tensor_tensor(aff_b[:, :], gnbT[:, :], ma[:, :], sub_op)

            out_re = out_d.ap().rearrange("(a p) d -> p a d", p=P)
            cp_i = 0
            for c in range(NQC):
                # split c=0 in halves so the first output-projection group
                # starts ~0.5us earlier off the groupnorm critical chain
                parts = ((0, QCH // 2), (QCH // 2, QCH)) if c == 0 \
                    else ((0, QCH),)
                for mt in range(2):
                    nrm = nrmp.tile([P, QCH], bf16, tag="nrm")
                    for lo, hi in parts:
                        osl = slice(c * QCH + lo, c * QCH + hi)
                        nc.scalar.activation(nrm[:, lo:hi], retT[:, mt, osl],
                                             ident,
                                             bias=aff_b[:, mt:mt + 1],
                                             scale=aff_a[:, mt:mt + 1])
                        nc.vector.tensor_tensor(gtdT[:, mt, osl],
                                                nrm[:, lo:hi],
                                                gateT[:, mt, osl], mul_op)
                for pair in range(2):
                    tt0 = 4 * c + 2 * pair
                    ob_t = outp.tile([P, 2, D], bf16, tag="ob")
                    for j in range(2):
                        tt = tt0 + j
                        for oc in range(2):
                            op_ps = psacc.tile([P, QCH], f32, tag="acc")
                            for kc in range(2):
                                nc.tensor.matmul(
                                    op_ps[:, :],
                                    lhsT=gtdT[:, kc, tt * P:(tt + 1) * P],
                                    rhs=woT[:, kc, oc * QCH:(oc + 1) * QCH],
                                    start=(kc == 0), stop=(kc == 1))
                            dst = ob_t[:, j, oc * QCH:(oc + 1) * QCH]
                            if cp_i % 2 == 0:
                                nc.vector.tensor_copy(dst, op_ps[:, :])
                            else:
                                nc.scalar.copy(dst, op_ps[:, :])
                            cp_i += 1
                    nc.sync.dma_start(out_re[:, tt0:tt0 + 2, :],
                                      ob_t[:, :, :])

    nc.compile()
    return nc


_PROGRAM_CACHE = {}


def _get_program(has_bv):
    if has_bv not in _PROGRAM_CACHE:
        _PROGRAM_CACHE[has_bv] = _build_program(has_bv)
    return _PROGRAM_CACHE[has_bv]


def kernel(**inputs):
    global LAST_EXEC_NS, LAST_RESULTS
    in_maps, has_bv = _host_prep(inputs)
    nc = _get_program(has_bv)
    trace = bool(int(os.environ.get("KERNEL_TRACE", "0")))
    kw = {}
    if trace:
        kw["trace"] = True
        kw["trace_cores"] = [int(c) for c in
                             os.environ.get("KERNEL_TRACE_CORES", "0").split(",")]
        td = os.environ.get("KERNEL_TRACE_DIR")
        if td:
            os.makedirs(td, exist_ok=True)
            kw["tmpdir"] = td
    res = run_bass_kernel_spmd(nc, in_maps, list(range(N_CORES)), **kw)
    LAST_EXEC_NS = res.exec_time_ns
    LAST_RESULTS = res
    bo = np.asarray(inputs["bo"], np.float32)
    out = np.zeros((B, S, D), np.float32)
    for core in range(N_CORES):
        out[core // HG] += res.results[core]["out"].astype(np.float32)
    out += bo[None, None, :]
    return out
